# revision 12
# baseline (speedup 1.0000x reference)
"""Trainium2 Bass kernel for the BKT (Bayesian Knowledge Tracing) scan model.

Contract: kernel(**inputs) takes the FULL unsharded inputs
  prev_kc   [512, 100] int32
  curr_kc   [512, 100] int32
  prev_corr [512, 100] int32
  kc_logits [5000, 5]  float32
and returns (probs [512, 100] f32, state_f [512, 5000] f32), matching the
jax reference (a lax.scan over T=100 with per-student gather/scatter into a
[B, 5000] skill-state table).

Sharding: pure data-parallel over students; 8 NeuronCores x 64 students.

Algorithm (per core, students on 64 partitions):
  The scan only ever touches, per student, the 99 kcs written
  (prev_kc[:,1:]) and the 100 kcs read (curr_kc).  We keep a compact
  199-slot value array V per student: slot i holds the *current* value of
  the kc named in slot i (write slots 0..98 = prev_kc[:,1:], read slots
  99..198 = curr_kc).  All slots start at sigmoid(logits[kc, 4]).
  A step t=1..99 reads s = V[:, t-1] (its own write slot already holds the
  current value of its kc), applies the BKT update
     v = (alpha*s + beta) / (gamma*s + delta)
  and writes v into EVERY slot whose kc equals this step's kc
  (copy_predicated with an equality mask).  The read slot 99+t then holds
  exactly state[ck_t] after the step, giving probs[:, t] in one fused
  multiply-add.  At the end every write slot holds the final value of its
  kc.

  Table rows are fetched with one dma_gather (rows padded to 256B in a
  DRAM scratch).  The final state is assembled in SBUF: local_scatter
  writes the final write-slot values (as int16 pairs) into a zeroed
  [64,5000] f32 image, then one copy_predicated merges it over the
  broadcast init row (the scattered values are probabilities > 0, so the
  f32 bit pattern doubles as the written-mask).
"""

import sys
import numpy as np

for _p in ("/opt/trn_rl_repo", "/opt/pypackages"):
    if _p not in sys.path:
        sys.path.insert(0, _p)

import concourse.bass as bass
import concourse.bacc as bacc
import concourse.mybir as mybir
import concourse.tile as tile
from concourse.bass import AP, IndirectOffsetOnAxis

F32 = mybir.dt.float32
I32 = mybir.dt.int32
I16 = mybir.dt.int16

B = 512
T = 100
K = 5000
NCORES = 8
BL = B // NCORES  # 64 students per core
NW = T - 1  # write steps 1..99 -> slots 0..98
NS = NW + T  # 199 slots
NIDX = 2 * T * BL  # 12800 gather items (write half padded by one col)
ESZ = 64  # gather row padded to 64 f32 = 256B

Alu = mybir.AluOpType

# local_scatter chunking of the state image viewed as [64, 10000] int16
CHUNK = 2016  # CHUNK*32 < 65536, even
NCHUNK = 5  # 4*2016 + 1936 = 10000


def build_program():
    nc = bacc.Bacc("TRN2", target_bir_lowering=False, debug=False)

    pk = nc.declare_dram_parameter("prev_kc", [BL, T], I32, isOutput=False)
    ck = nc.declare_dram_parameter("curr_kc", [BL, T], I32, isOutput=False)
    pc = nc.declare_dram_parameter("prev_corr", [BL, T], I32, isOutput=False)
    lg = nc.declare_dram_parameter("kc_logits", [K, 5], F32, isOutput=False)
    probs_out = nc.declare_dram_parameter("probs_out", [BL, T], F32, isOutput=True)
    state_out = nc.declare_dram_parameter("state_out", [BL, K], F32, isOutput=True)

    ptblp = nc.dram_tensor("ptblp", [K, ESZ], F32)  # padded sigmoid table
    initrow = nc.dram_tensor("initrow", [1, K], F32)  # sigmoid(logits[:,4])

    with tile.TileContext(nc) as tc:
        with tc.tile_pool(name="p", bufs=1) as pool:
            # ---- sigmoid of the full table ----
            lg_v = lg[:].rearrange("(p a) c -> p (a c)", p=125)
            sig = pool.tile([125, 200], F32, tag="sig")
            nc.sync.dma_start(sig[:], lg_v)
            nc.scalar.activation(sig[:], sig[:], mybir.ActivationFunctionType.Sigmoid)
            # scatter the 5 real columns into the 64-wide padded table rows
            nc.sync.dma_start(ptblp[:].rearrange("(p a) c -> p a c", p=125)[:, :, 0:5],
                              sig[:].rearrange("p (a c) -> p a c", c=5))
            # init row = col 4 of the sigmoid table -> contiguous [1, 5000]
            sig3 = sig[:].rearrange("p (a c) -> p a c", c=5)
            nc.sync.dma_start(
                initrow[:].rearrange("o (p a) -> (o p) a", p=125), sig3[:, :, 4]
            )
            # broadcast init row over 64 partitions in SBUF (final-state base)
            init_bc = pool.tile([BL, K], F32, tag="init_bc")
            nc.sync.dma_start(init_bc[:], AP(initrow[:].tensor, 0, [[0, BL], [1, K]]))

            # ---- inputs ----
            kcin = pool.tile([BL, NS], I32, tag="kcin")  # slot kc ids
            nc.sync.dma_start(kcin[:, 0:NW], pk[:, 1:T])
            nc.sync.dma_start(kcin[:, NW:NS], ck[:, 0:T])
            corr_i = pool.tile([BL, NW], I32, tag="corr_i")
            nc.sync.dma_start(corr_i[:], pc[:, 1:T])
            corr = pool.tile([BL, NW], F32, tag="corr")
            nc.vector.tensor_copy(corr[:], corr_i[:])
            slotkc = pool.tile([BL, NS], F32, tag="slotkc")
            nc.vector.tensor_copy(slotkc[:], kcin[:])

            # ---- dma_gather index layout ----
            # gather item i = j*128 + p: partitions 0..63 = (student p, read
            # slot j=0..99 -> curr_kc[p, j]); partitions 64..127 = (student
            # p-64, write slot j -> prev_kc[p-64, j+1]), j=99 padded with 0.
            # idx i lives at [i % 16, i // 16] of a [16, NIDX/16] i16 block,
            # replicated across all 8 16-partition groups.
            ck16 = pool.tile([BL, T], I16, tag="ck16")
            nc.vector.tensor_copy(ck16[:], kcin[:, NW:NS])
            pk16 = pool.tile([BL, T], I16, tag="pk16")
            nc.vector.memset(pk16[:], 0)
            nc.vector.tensor_copy(pk16[:, 0:NW], kcin[:, 0:NW])

            gidx = pool.tile([128, NIDX // 16], I16, tag="gidx")
            # gidx[r, j*8 + b] = kc(p = b*16 + r, j); one DMA per 16-row group
            for b_ in range(4):
                nc.sync.dma_start(
                    gidx[0:16, b_ : NIDX // 16 : 8], ck16[b_ * 16 : (b_ + 1) * 16, :]
                )
            for b_ in range(4):
                nc.sync.dma_start(
                    gidx[0:16, 4 + b_ : NIDX // 16 : 8],
                    pk16[b_ * 16 : (b_ + 1) * 16, :],
                )
            # replicate the [16, .] block to all 8 groups by doubling
            nc.sync.dma_start(gidx[16:32, :], gidx[0:16, :])
            nc.sync.dma_start(gidx[32:64, :], gidx[0:32, :])
            nc.sync.dma_start(gidx[64:128, :], gidx[0:64, :])

            # ---- the gather: out64[p, j, 0:5] = table rows ----
            out64 = pool.tile([128, T, ESZ], F32, tag="out64")
            nc.gpsimd.dma_gather(
                out_ap=out64[:],
                in_ap=ptblp[:],
                idxs_ap=gidx[:],
                num_idxs=NIDX,
                num_idxs_reg=NIDX,
                elem_size=ESZ,
                single_packet=False,
            )
            # move write-slot rows (partitions 64..127) onto partitions 0..63
            wrows = pool.tile([BL, NW, 5], F32, tag="wrows")
            nc.sync.dma_start(wrows[:], out64[64:128, 0:NW, 0:5])

            L = wrows[:, :, 0]
            Fg = wrows[:, :, 1]
            p2 = wrows[:, :, 2]
            p3 = wrows[:, :, 3]
            g0 = out64[0:BL, :, 2]
            g1 = out64[0:BL, :, 3]

            # ---- per-step Moebius coefficients ----
            # c in {0,1}: a = (1-c)(1-p2)+c*p2,  b = (1-c)(1-p3)+c*p3
            t1 = pool.tile([BL, NW], F32, tag="t1")
            t2 = pool.tile([BL, NW], F32, tag="t2")
            av = pool.tile([BL, NW], F32, tag="av")  # a == delta
            bv = pool.tile([BL, NW], F32, tag="bv")
            nc.vector.tensor_scalar(t1[:], p2, 2.0, -1.0, Alu.mult, Alu.add)
            nc.vector.tensor_scalar(t2[:], p2, -1.0, 1.0, Alu.mult, Alu.add)
            nc.vector.tensor_tensor(out=av[:], in0=corr[:], in1=t1[:], op=Alu.mult)
            nc.vector.tensor_tensor(out=av[:], in0=av[:], in1=t2[:], op=Alu.add)
            nc.vector.tensor_scalar(t1[:], p3, 2.0, -1.0, Alu.mult, Alu.add)
            nc.vector.tensor_scalar(t2[:], p3, -1.0, 1.0, Alu.mult, Alu.add)
            nc.vector.tensor_tensor(out=bv[:], in0=corr[:], in1=t1[:], op=Alu.mult)
            nc.vector.tensor_tensor(out=bv[:], in0=bv[:], in1=t2[:], op=Alu.add)

            # v = (alpha*s + beta) / (gamma*s + delta)
            # alpha = b*(1-F) - L*a ; beta = L*a ; gamma = b - a ; delta = a
            beta = pool.tile([BL, NW], F32, tag="beta")
            alpha = pool.tile([BL, NW], F32, tag="alpha")
            gamma = pool.tile([BL, NW], F32, tag="gamma")
            nc.vector.tensor_tensor(out=beta[:], in0=L, in1=av[:], op=Alu.mult)
            nc.vector.tensor_scalar(t1[:], Fg, -1.0, 1.0, Alu.mult, Alu.add)
            nc.vector.tensor_tensor(out=t1[:], in0=bv[:], in1=t1[:], op=Alu.mult)
            nc.vector.tensor_tensor(out=alpha[:], in0=t1[:], in1=beta[:], op=Alu.subtract)
            nc.vector.tensor_tensor(out=gamma[:], in0=bv[:], in1=av[:], op=Alu.subtract)

            gd = pool.tile([BL, T], F32, tag="gd")  # g1 - g0
            nc.vector.tensor_tensor(out=gd[:], in0=g1, in1=g0, op=Alu.subtract)

            # ---- V slots: current value per slot kc, start at init ----
            V = pool.tile([BL, NS], F32, tag="V")
            nc.vector.tensor_copy(V[:, 0:NW], wrows[:, :, 4])
            nc.vector.tensor_copy(V[:, NW:NS], out64[0:BL, :, 4])

            probs = pool.tile([BL, T], F32, tag="probs")
            # t = 0: cs0 = init[ck_0] = V[:, 99]
            nc.vector.tensor_scalar(
                probs[:, 0:1], V[:, NW : NW + 1], gd[:, 0:1], g0[:, 0:1],
                Alu.mult, Alu.add,
            )

            maskb = pool.tile([BL, NS], mybir.dt.uint8, tag="maskb")
            numb = pool.tile([BL, 1], F32, tag="numb")
            denb = pool.tile([BL, 1], F32, tag="denb")
            recb = pool.tile([BL, 1], F32, tag="recb")
            vb = pool.tile([BL, 1], F32, tag="vb")

            for t in range(1, T):
                i = t - 1  # write-slot / coefficient index
                s = V[:, i : i + 1]
                nc.vector.tensor_scalar(
                    maskb[:], slotkc[:], slotkc[:, i : i + 1], None, Alu.is_equal
                )
                nc.vector.tensor_scalar(
                    numb[:], s, alpha[:, i : i + 1], beta[:, i : i + 1],
                    Alu.mult, Alu.add,
                )
                nc.vector.tensor_scalar(
                    denb[:], s, gamma[:, i : i + 1], av[:, i : i + 1],
                    Alu.mult, Alu.add,
                )
                nc.vector.reciprocal(recb[:], denb[:])
                nc.vector.tensor_tensor(out=vb[:], in0=numb[:], in1=recb[:], op=Alu.mult)
                nc.vector.copy_predicated(V[:], maskb[:], vb[:].to_broadcast([BL, NS]))
                nc.vector.tensor_scalar(
                    probs[:, t : t + 1], V[:, NW + t : NW + t + 1],
                    gd[:, t : t + 1], g0[:, t : t + 1], Alu.mult, Alu.add,
                )

            nc.sync.dma_start(probs_out[:], probs[:])

            # ---- final state: local_scatter V write slots over init row ----
            # int16-pair indices: 2*w and 2*w+1 into the [64, 10000] i16 view
            w2 = pool.tile([BL, NW], F32, tag="w2")
            nc.vector.tensor_scalar(w2[:], slotkc[:, 0:NW], 2.0, None, Alu.mult)
            scat = pool.tile([BL, K], F32, tag="scat")
            scat16 = scat[:].bitcast(I16)  # [64, 10000]
            vdata = V[:, 0:NW].bitcast(I16)  # [64, 198]
            basec = pool.tile([BL, NW], F32, tag="basec")
            mvalid = pool.tile([BL, NW], F32, tag="mvalid")
            madj = pool.tile([BL, NW], F32, tag="madj")
            idx16 = pool.tile([BL, 2 * NW], I16, tag="idx16")
            idxf = pool.tile([BL, 2 * NW], F32, tag="idxf")
            for c in range(NCHUNK):
                lo = c * CHUNK
                ne = min(CHUNK, 2 * K - lo)
                # base = 2*w - lo ; push >= ne out-of-range to negative
                nc.vector.tensor_scalar(
                    basec[:], w2[:], 1.0, float(-lo), Alu.mult, Alu.add
                )
                nc.vector.tensor_scalar(
                    mvalid[:], basec[:], float(ne - 1), None, Alu.is_le
                )
                nc.vector.tensor_scalar(
                    madj[:], mvalid[:], 20000.0, -20000.0, Alu.mult, Alu.add
                )
                nc.vector.tensor_tensor(
                    out=basec[:], in0=basec[:], in1=madj[:], op=Alu.add
                )
                nc.vector.tensor_scalar(
                    idxf[:, 0 : 2 * NW : 2], basec[:], 1.0, 0.0, Alu.mult, Alu.add
                )
                nc.vector.tensor_scalar(
                    idxf[:, 1 : 2 * NW : 2], basec[:], 1.0, 1.0, Alu.mult, Alu.add
                )
                nc.vector.tensor_copy(idx16[:], idxf[:])
                nc.gpsimd.local_scatter(
                    out_ap=scat16[:, lo : lo + ne],
                    data_ap=vdata,
                    idxs_ap=idx16[:],
                    channels=BL,
                    num_elems=ne,
                    num_idxs=2 * NW,
                )
            # merge over init: nonzero f32 bits of scat = "written" mask
            nc.vector.copy_predicated(init_bc[:], scat[:].bitcast(I32), scat[:])
            nc.sync.dma_start(state_out[:], init_bc[:])

    nc.compile()
    return nc


_CACHE = {}


def _get_nc():
    if "nc" not in _CACHE:
        _CACHE["nc"] = build_program()
    return _CACHE["nc"]


def _run(prev_kc, curr_kc, prev_corr, kc_logits, trace=False):
    from concourse.bass_utils import run_bass_kernel_spmd

    nc = _get_nc()
    prev_kc = np.ascontiguousarray(np.asarray(prev_kc, dtype=np.int32))
    curr_kc = np.ascontiguousarray(np.asarray(curr_kc, dtype=np.int32))
    prev_corr = np.ascontiguousarray(np.asarray(prev_corr, dtype=np.int32))
    kc_logits = np.ascontiguousarray(np.asarray(kc_logits, dtype=np.float32))

    in_maps = []
    for c in range(NCORES):
        sl = slice(c * BL, (c + 1) * BL)
        in_maps.append(
            {
                "prev_kc": prev_kc[sl],
                "curr_kc": curr_kc[sl],
                "prev_corr": prev_corr[sl],
                "kc_logits": kc_logits,
            }
        )
    res = run_bass_kernel_spmd(nc, in_maps, list(range(NCORES)), trace=trace)
    _CACHE["last_results"] = res
    probs = np.concatenate([res.results[c]["probs_out"] for c in range(NCORES)], axis=0)
    state = np.concatenate([res.results[c]["state_out"] for c in range(NCORES)], axis=0)
    return probs, state


def kernel(prev_kc, curr_kc, prev_corr, kc_logits):
    return _run(prev_kc, curr_kc, prev_corr, kc_logits, trace=False)
